# revision 28
# baseline (speedup 1.0000x reference)
"""MHA Bass kernel for TRN2, 8 NeuronCores.

Sharding: data-parallel on batch (2) x tensor-parallel on heads (4 groups of 4
heads). Core c handles batch c//4 and heads 4*(c%4)..4*(c%4)+3 (columns
m0=256*(c%4)).

v2 design vs baseline:
- Inputs pre-cast to bf16 and pre-transposed on host -> xT arrives [1024,2048]
  bf16; direct chunked loads, no on-device cast or SBUF transposes.
- LayerNorm folded into projections (gamma on host, mean/var on device via
  ones-matmuls; rstd via ACT Sqrt + DVE reciprocal_approx_fast).
- v-heads transposed into natural layout via DMA-transpose (no PE transposes).
- Attention with transposed scores (S^T); exp batched N=1024 per ACT instr
  (2 PSUM banks); softmax denominators ride the O-matmul as a ones-column.
- q/g projections interleaved with attention per 512-token tile; out-proj and
  per-tile ReduceScatter pipelined under the next tile's attention.
"""
import numpy as np

B, LQ, D = 2, 2048, 1024
NHEAD, DHEAD = 16, 64
NC = 8
GPC = 4              # cores per batch group
MPC = 256            # output cols per core
N_DCH = D // 128     # 8 d-chunks
N_TT = LQ // 512     # 4 token tiles of 512
N_SCH = LQ // 128    # 16 key chunks

_NC_CACHE = [None]
DEBUG_DUMPS = False


def _build():
    import concourse.bacc as bacc
    import concourse.mybir as mybir
    from concourse import tile

    f32, bf16 = mybir.dt.float32, mybir.dt.bfloat16
    AF = mybir.ActivationFunctionType
    MUL, ADD, SUB = mybir.AluOpType.mult, mybir.AluOpType.add, mybir.AluOpType.subtract

    nc = bacc.Bacc("TRN2", target_bir_lowering=False, debug=False, num_devices=NC)

    xqT = nc.dram_tensor("xqT", [D, LQ], bf16, kind="ExternalInput").ap()
    xkT = nc.dram_tensor("xkT", [D, LQ], bf16, kind="ExternalInput").ap()
    xvT = nc.dram_tensor("xvT", [D, LQ], bf16, kind="ExternalInput").ap()
    wqT = nc.dram_tensor("wqT", [D, MPC], bf16, kind="ExternalInput").ap()
    wkT = nc.dram_tensor("wkT", [D, MPC], bf16, kind="ExternalInput").ap()
    wvT = nc.dram_tensor("wvT", [D, MPC], bf16, kind="ExternalInput").ap()
    wgT = nc.dram_tensor("wgT", [D, MPC], bf16, kind="ExternalInput").ap()
    woT = nc.dram_tensor("woT", [MPC, D], bf16, kind="ExternalInput").ap()
    mucq = nc.dram_tensor("mucq", [1, MPC], bf16, kind="ExternalInput").ap()
    muck = nc.dram_tensor("muck", [1, MPC], bf16, kind="ExternalInput").ap()
    mucv = nc.dram_tensor("mucv", [1, MPC], bf16, kind="ExternalInput").ap()
    mucg = nc.dram_tensor("mucg", [1, MPC], bf16, kind="ExternalInput").ap()
    bq_d = nc.dram_tensor("bq", [MPC], f32, kind="ExternalInput").ap()
    bk_d = nc.dram_tensor("bk", [MPC], f32, kind="ExternalInput").ap()
    bv_d = nc.dram_tensor("bv", [MPC], f32, kind="ExternalInput").ap()
    bg_d = nc.dram_tensor("bgt", [MPC], f32, kind="ExternalInput").ap()
    out_d = nc.dram_tensor("out", [MPC, LQ], bf16, kind="ExternalOutput").ap()
    wu_d = nc.dram_tensor("wu", [128, 128], f32, kind="ExternalOutput").ap()
    dbg = {}
    if DEBUG_DUMPS:
        for nm, shp in (("d_khT", [128, 2, LQ]), ("d_qhT", [128, 2, LQ]),
                        ("d_gT", [128, 2, LQ]), ("d_ygT", [128, 2, LQ]),
                        ("d_vaug", [128, N_SCH, 4, 65])):
            dbg[nm] = nc.dram_tensor(nm, shp, mybir.dt.bfloat16,
                                     kind="ExternalOutput").ap()
        dbg["d_rq"] = nc.dram_tensor("d_rq", [128, LQ], f32, kind="ExternalOutput").ap()
        dbg["d_outb"] = nc.dram_tensor("d_outb", [N_TT, D, 512], mybir.dt.bfloat16,
                                       kind="ExternalOutput").ap()

    with tile.TileContext(nc) as tc:
        import contextlib
        es = contextlib.ExitStack()
        with es:
            const = es.enter_context(tc.tile_pool(name="const", bufs=1))
            persist = es.enter_context(tc.tile_pool(name="persist", bufs=1))

            ones = const.tile([128, 128], bf16)
            nc.gpsimd.memset(ones[:, :], 1.0)
            eps_t = const.tile([128, 1], f32)
            nc.gpsimd.memset(eps_t[:, :], 1e-5)

            wts, mucs, biases = {}, {}, {}
            # persistent activation outputs
            qhT = persist.tile([128, 2, LQ], bf16, tag="qhT")
            khT = persist.tile([128, 2, LQ], bf16, tag="khT")
            gT = persist.tile([128, 2, LQ], bf16, tag="gT")
            ygT = persist.tile([128, 2, LQ], bf16, tag="ygT")
            vaug = persist.tile([128, N_SCH, 4, 65], bf16, tag="vaug")
            # contiguous memset; v-transposes later overwrite cols 0:64, col 64 stays 1.0
            nc.gpsimd.memset(vaug[:, :, :, :], 1.0)

            xpool = es.enter_context(tc.tile_pool(name="xp", bufs=2))
            statp = es.enter_context(tc.tile_pool(name="stat", bufs=2))
            vhp = es.enter_context(tc.tile_pool(name="vhp", bufs=1))
            scr = es.enter_context(tc.tile_pool(name="scr", bufs=2))
            # shared [128,512] f32 PSUM tiles: proj pp, out-proj po, bcast bc
            ps_w = es.enter_context(tc.tile_pool(name="ps_w", bufs=2, space="PSUM"))

            def load_x(dr):
                """[1024, 2048] bf16 DRAM -> xT [128, 8, 2048] (per-chunk DMAs)."""
                xT = xpool.tile([128, N_DCH, LQ], bf16, tag="xT")
                for j in range(N_DCH):
                    nc.sync.dma_start(out=xT[:, j, :],
                                      in_=dr[128 * j:128 * (j + 1), :])
                return xT

            def stats(xT, ps_s):
                """r_rep [128,2048] f32 = rstd (broadcast); mu [1,2048] bf16."""
                r_rep = statp.tile([128, LQ], f32, tag="r_rep")
                mu = statp.tile([1, LQ], bf16, tag="mu")
                for tt in range(N_TT):
                    sl = slice(512 * tt, 512 * (tt + 1))
                    s1 = ps_s.tile([128, 512], f32, tag="s1")
                    s2 = ps_s.tile([128, 512], f32, tag="s2")
                    for j in range(N_DCH):
                        nc.tensor.matmul(s1[:, :], ones[:, :], xT[:, j, sl],
                                         start=(j == 0), stop=(j == N_DCH - 1))
                    for j in range(N_DCH):
                        sq = scr.tile([128, 512], bf16, tag="sq")
                        nc.vector.tensor_mul(sq[:, :], xT[:, j, sl], xT[:, j, sl])
                        nc.tensor.matmul(s2[:, :], ones[:, :], sq[:, :],
                                         start=(j == 0), stop=(j == N_DCH - 1))
                    s1s = scr.tile([128, 512], f32, tag="s1s")
                    nc.vector.tensor_copy(s1s[:, :], s1[:, :])
                    t1 = scr.tile([128, 512], f32, tag="t1")
                    nc.vector.tensor_mul(t1[:, :], s1s[:, :], s1s[:, :])
                    t2 = scr.tile([128, 512], f32, tag="t2")
                    nc.vector.scalar_tensor_tensor(t2[:, :], s2[:, :], 1024.0, t1[:, :], MUL, SUB)
                    t3 = scr.tile([128, 512], f32, tag="t3")
                    nc.scalar.activation(t3[:, :], t2[:, :], AF.Sqrt,
                                         bias=eps_t[:, :], scale=1.0 / (1024.0 * 1024.0))
                    nc.vector.reciprocal_approx_fast(r_rep[:, sl], t3[:, :])
                    nc.vector.tensor_scalar(mu[0:1, sl], s1s[0:1, :], 1.0 / 1024.0, None, op0=MUL)
                return r_rep, mu

            def project_tt(xT, wkey, muckey, mu, r_rep, out_t, tt, sigmoid=False):
                """out_t[:, mc, tt-slice] = ((x-mu)@W'^T)*rstd [+bias / sigmoid]"""
                w = wts[wkey]
                mc_t = mucs[muckey]
                bias = biases[muckey]
                sl = slice(512 * tt, 512 * (tt + 1))
                for mc in range(2):
                    pp = ps_w.tile([128, 512], f32, tag="w")
                    for j in range(N_DCH):
                        nc.tensor.matmul(pp[:, :], w[:, j, 128 * mc:128 * (mc + 1)],
                                         xT[:, j, sl], start=(j == 0), stop=False)
                    nc.tensor.matmul(pp[:, :], mc_t[:, 128 * mc:128 * (mc + 1)],
                                     mu[0:1, sl], start=False, stop=True)
                    if sigmoid:
                        # sigmoid(x+b) = 0.5*tanh(0.5*(x+b)) + 0.5 -- tanh shares
                        # the exp table set (no ACT table thrash in attention)
                        tmp = scr.tile([128, 512], f32, tag="ptmp")
                        nc.vector.tensor_mul(tmp[:, :], pp[:, :], r_rep[:, sl])
                        tnh = scr.tile([128, 512], f32, tag="tnh")
                        nc.scalar.activation(tnh[:, :], tmp[:, :], AF.Tanh,
                                             bias=bias[:, mc:mc + 1], scale=0.5)
                        nc.vector.tensor_scalar(out_t[:, mc, sl], tnh[:, :],
                                                0.5, 0.5, op0=MUL, op1=ADD)
                    else:
                        nc.vector.scalar_tensor_tensor(
                            out_t[:, mc, sl], pp[:, :], bias[:, mc:mc + 1],
                            r_rep[:, sl], ADD, MUL)

            with tc.tile_pool(name="ps_s", bufs=2, space="PSUM") as ps_s:
                # ---- k (loaded first so stats can start ASAP) ----
                xT = load_x(xkT)

                # PE warm-up burst during the load window: ~60 dense matmuls
                # engage the HAM clock gate (K=8/8) before real work arrives
                wu = ps_w.tile([128, 512], f32, tag="w")
                for i in range(60):
                    nc.tensor.matmul(wu[:, 0:128], ones[:, :], ones[:, :],
                                     start=(i == 0), stop=(i == 59))
                wus = scr.tile([128, 128], f32, tag="wus")
                nc.vector.tensor_copy(wus[:, :], wu[:, 0:128])
                nc.sync.dma_start(out=wu_d[:, :], in_=wus[:, :])

                # weights: [128, 8, 256] layouts (d-chunk, cols)
                for nm, dr in (("q", wqT), ("k", wkT), ("v", wvT), ("g", wgT)):
                    t = const.tile([128, N_DCH, MPC], bf16, tag=f"w{nm}")
                    nc.sync.dma_start(out=t[:, :, :],
                                      in_=dr.rearrange("(c p) m -> p c m", p=128))
                    wts[nm] = t
                wo_t = const.tile([128, 2, D], bf16)
                nc.sync.dma_start(out=wo_t[:, :, :],
                                  in_=woT.rearrange("(c p) d -> p c d", p=128))
                for nm, dr in (("q", mucq), ("k", muck), ("v", mucv), ("g", mucg)):
                    t = const.tile([1, MPC], bf16, tag=f"muc{nm}")
                    nc.sync.dma_start(out=t[:, :], in_=dr[:, :])
                    mucs[nm] = t
                for nm, dr in (("q", bq_d), ("k", bk_d), ("v", bv_d), ("g", bg_d)):
                    t = const.tile([128, 2], f32, tag=f"b{nm}")
                    nc.sync.dma_start(out=t[:, :],
                                      in_=dr.rearrange("(c p) -> p c", p=128))
                    biases[nm] = t

                r_rep, mu = stats(xT, ps_s)
                for tt in range(N_TT):
                    project_tt(xT, "k", "k", mu, r_rep, khT, tt)
                # ---- v ----
                xTv = load_x(xvT)
                r_rep, mu = stats(xTv, ps_s)
                vhT = vhp.tile([128, 2, LQ], bf16, tag="vhT")
                for tt in range(N_TT):
                    project_tt(xTv, "v", "v", mu, r_rep, vhT, tt)
                # v -> natural layout [k-token, dh]: DMA transpose needs a
                # contiguous output, so bounce via scratch then strided DVE copy
                with tc.tile_pool(name="vtrp", bufs=1) as vtrp:
                    for mc in range(2):
                        for hb in range(2):
                            vtr = vtrp.tile([128, N_SCH, 64], bf16, tag="vtr")
                            nc.sync.dma_start_transpose(
                                out=vtr[:, :, :],
                                in_=vhT[64 * hb:64 * (hb + 1), mc, :])
                            nc.vector.tensor_copy(vaug[:, :, 2 * mc + hb, 0:64],
                                                  vtr[:, :, :])
                # ---- q (stats only; projections interleaved with attention) ----
                xTq = load_x(xqT)
                r_q, mu_q = stats(xTq, ps_s)

            # ---- attention + out-proj + RS, token-tile outer ----
            att = es.enter_context(tc.tile_pool(name="att", bufs=2))
            attn = es.enter_context(tc.tile_pool(name="attn", bufs=2))
            ps_st = es.enter_context(tc.tile_pool(name="ps_st", bufs=2, space="PSUM"))
            ps_o = es.enter_context(tc.tile_pool(name="ps_o", bufs=1, space="PSUM"))
            od = es.enter_context(tc.tile_pool(name="od", bufs=2))
            dram_p = es.enter_context(tc.tile_pool(name="dram", bufs=1, space="DRAM"))
            outb = dram_p.tile([N_TT, D, 512], bf16, tag="outb")
            outrs = dram_p.tile([N_TT, MPC, 512], bf16, tag="outrs")

            for th in range(N_TT):
                slth = slice(512 * th, 512 * (th + 1))
                # project this tile's q and g
                project_tt(xTq, "q", "q", mu_q, r_q, qhT, th)
                project_tt(xTq, "g", "g", mu_q, r_q, gT, th, sigmoid=True)
                for hp in range(2):
                    o4 = ps_o.tile([65, 2, 512], f32, tag="o4")
                    for s in range(N_SCH):
                        st2 = ps_st.tile([128, 2, 512], f32, tag="st2")
                        for hb in range(2):
                            nc.tensor.matmul(
                                st2[:, hb, :],
                                khT[64 * hb:64 * (hb + 1), hp, 128 * s:128 * (s + 1)],
                                qhT[64 * hb:64 * (hb + 1), hp, slth],
                                start=True, stop=True)
                        pt = att.tile([128, 2, 512], bf16, tag="pt")
                        nc.scalar.activation(pt[:, :, :], st2[:, :, :], AF.Exp, scale=0.125)
                        for hb in range(2):
                            nc.tensor.matmul(o4[:, hb, :],
                                             vaug[:, s, 2 * hp + hb, :], pt[:, hb, :],
                                             start=(s == 0), stop=(s == N_SCH - 1))
                    # normalize + gate: broadcast denominator via K=1 matmul,
                    # reciprocal at base partition 0 (approx_fast is broken at
                    # nonzero base partitions on HW)
                    for hb in range(2):
                        r0 = 64 * hb
                        li_b = attn.tile([65, 512], bf16, tag="lib")
                        nc.vector.tensor_copy(li_b[64:65, :], o4[64:65, hb, :])
                        bc = ps_w.tile([128, 512], f32, tag="w")
                        nc.tensor.matmul(bc[0:64, :], ones[64:65, 0:64],
                                         li_b[64:65, :], start=True, stop=True)
                        bcs = attn.tile([64, 512], f32, tag="bcs")
                        nc.vector.tensor_copy(bcs[:, :], bc[0:64, :])
                        bcr = attn.tile([64, 512], f32, tag="bcr")
                        nc.vector.reciprocal_approx_fast(bcr[:, :], bcs[:, :])
                        tmp2 = attn.tile([128, 512], f32, tag="tmp2")
                        nc.vector.tensor_mul(tmp2[0:64, :], o4[0:64, hb, :], bcr[:, :])
                        if r0 != 0:
                            # partition shift 0->64 via SBUF->SBUF DMA
                            nc.gpsimd.dma_start(out=tmp2[64:128, :], in_=tmp2[0:64, :])
                        nc.vector.tensor_mul(ygT[r0:r0 + 64, hp, slth],
                                             tmp2[r0:r0 + 64, :],
                                             gT[r0:r0 + 64, hp, slth])
                # out-proj for this token tile
                for nk in range(N_DCH):
                    po = ps_w.tile([128, 512], f32, tag="w")
                    for mc in range(2):
                        nc.tensor.matmul(po[:, :], wo_t[:, mc, 128 * nk:128 * (nk + 1)],
                                         ygT[:, mc, slth],
                                         start=(mc == 0), stop=(mc == 1))
                    ot = od.tile([128, 512], bf16, tag="ot")
                    nc.vector.tensor_copy(ot[:, :], po[:, :])
                    nc.sync.dma_start(out=outb[th, 128 * nk:128 * (nk + 1), :],
                                      in_=ot[:, :])
                nc.gpsimd.collective_compute(
                    "ReduceScatter", ADD,
                    replica_groups=[[0, 1, 2, 3], [4, 5, 6, 7]],
                    ins=[outb[th, :, :].opt()],
                    outs=[outrs[th, :, :].opt()],
                )
                nc.sync.dma_start(out=out_d[:, 512 * th:512 * (th + 1)],
                                  in_=outrs[th, :, :])
            if DEBUG_DUMPS:
                nc.sync.dma_start(out=dbg["d_khT"][:, :, :], in_=khT[:, :, :])
                nc.sync.dma_start(out=dbg["d_qhT"][:, :, :], in_=qhT[:, :, :])
                nc.sync.dma_start(out=dbg["d_gT"][:, :, :], in_=gT[:, :, :])
                nc.sync.dma_start(out=dbg["d_ygT"][:, :, :], in_=ygT[:, :, :])
                nc.sync.dma_start(out=dbg["d_vaug"][:, :, :, :], in_=vaug[:, :, :, :])
                nc.sync.dma_start(out=dbg["d_rq"][:, :], in_=r_q[:, :])
                nc.sync.dma_start(out=dbg["d_outb"][:, :, :], in_=outb[:, :, :])

    nc.compile()
    return nc


def kernel(q, k, v, qln_g, qln_b, kvln_g, kvln_b, Wq, Wk, Wv, Wg, bg, Wo):
    import concourse.mybir as mybir
    from concourse import bass_utils

    bf16 = mybir.dt.np(mybir.dt.bfloat16)
    q = np.asarray(q, np.float32)
    k = np.asarray(k, np.float32)
    v = np.asarray(v, np.float32)
    qln_g = np.asarray(qln_g, np.float32)
    qln_b = np.asarray(qln_b, np.float32)
    kvln_g = np.asarray(kvln_g, np.float32)
    kvln_b = np.asarray(kvln_b, np.float32)
    Wq, Wk, Wv = np.asarray(Wq, np.float32), np.asarray(Wk, np.float32), np.asarray(Wv, np.float32)
    Wg, Wo = np.asarray(Wg, np.float32), np.asarray(Wo, np.float32)
    bg = np.asarray(bg, np.float32)

    # fold LN gamma into weights; beta into bias vectors
    Wqp, Wgp = Wq * qln_g[None, :], Wg * qln_g[None, :]
    Wkp, Wvp = Wk * kvln_g[None, :], Wv * kvln_g[None, :]
    bq_f, bk_f, bv_f = Wq @ qln_b, Wk @ kvln_b, Wv @ kvln_b
    bg_f = (Wg @ qln_b + bg) * 0.5  # pre-halved for the tanh-form sigmoid

    if _NC_CACHE[0] is None:
        _NC_CACHE[0] = _build()
    nc = _NC_CACHE[0]

    # pre-transposed bf16 inputs, shared per batch
    xT = {}
    for beta in range(B):
        xT[("q", beta)] = np.ascontiguousarray(q[beta].T).astype(bf16)
        xT[("k", beta)] = np.ascontiguousarray(k[beta].T).astype(bf16)
        xT[("v", beta)] = np.ascontiguousarray(v[beta].T).astype(bf16)

    in_maps = []
    for c in range(NC):
        beta, g = c // GPC, c % GPC
        sl = slice(MPC * g, MPC * (g + 1))
        in_maps.append({
            "xqT": xT[("q", beta)], "xkT": xT[("k", beta)], "xvT": xT[("v", beta)],
            "wqT": Wqp[sl, :].T.astype(bf16), "wkT": Wkp[sl, :].T.astype(bf16),
            "wvT": Wvp[sl, :].T.astype(bf16), "wgT": Wgp[sl, :].T.astype(bf16),
            "woT": Wo[:, sl].T.astype(bf16),
            "mucq": -Wqp[sl, :].sum(1)[None, :].astype(bf16),
            "muck": -Wkp[sl, :].sum(1)[None, :].astype(bf16),
            "mucv": -Wvp[sl, :].sum(1)[None, :].astype(bf16),
            "mucg": -Wgp[sl, :].sum(1)[None, :].astype(bf16),
            "bq": bq_f[sl], "bk": bk_f[sl], "bv": bv_f[sl], "bgt": bg_f[sl],
        })
    global _last_in_maps
    _last_in_maps = in_maps
    res = bass_utils.run_bass_kernel_spmd(nc, in_maps, core_ids=list(range(NC)))
    out = np.empty((B, LQ, D), np.float32)
    for beta in range(B):
        for g in range(GPC):
            out[beta, :, MPC * g:MPC * (g + 1)] = \
                res.results[GPC * beta + g]["out"].astype(np.float32).T
    return out
